# revision 1
# baseline (speedup 1.0000x reference)
"""AttentionPool2d Trainium2 kernel, 8-core batch-data-parallel.

Math (reference returns only query position 0):
  xf = [x.flat, mean] + pos  (permuted: cols 0..255 spatial, col 256 = mean tok)
  q0 = W_q @ xf_m + b_q                 (the only query needed)
  u_h = W_k_h^T q0_h  (folds W_k into the query; k never materialized)
  logits = (1/8) u^T xf ; w = softmax(logits)
  y = xf @ w'^T (+ pos-terms)           (w' = w_sp + w_m/256 absorbs mean token)
  a0_h = W_v_h y_h + b_v ; out = w_c a0 + b_c
"""
import sys, types
sys.path.insert(0, "/opt/trn_rl_repo")
import numpy as np
import ml_dtypes
from contextlib import ExitStack

from concourse import bacc, tile, mybir
import concourse.bass as bass
from concourse import masks
from concourse.bass_utils import run_bass_kernel_spmd

P = 128
B, C, S2, L = 64, 1024, 256, 257
NH, CHD = 16, 64
NCORE, BPC, CT = 8, 8, 8          # cores, batches/core, c-tiles
F32R = mybir.dt.float32r
F32 = mybir.dt.float32
BF16 = mybir.dt.bfloat16
AF = mybir.ActivationFunctionType
SCALE2 = 1.0 / 8.0                 # (1/ch^0.25)^2 folded into u


def _body(ctx: ExitStack, tc, d):
    nc = tc.nc
    const = ctx.enter_context(tc.tile_pool(name="const", bufs=1))
    wbig = ctx.enter_context(tc.tile_pool(name="wbig", bufs=2))
    wsml = ctx.enter_context(tc.tile_pool(name="wsml", bufs=1))
    xres = ctx.enter_context(tc.tile_pool(name="xres", bufs=1))
    xtp = ctx.enter_context(tc.tile_pool(name="xtp", bufs=1))
    wbf = ctx.enter_context(tc.tile_pool(name="wbf", bufs=2))
    work = ctx.enter_context(tc.tile_pool(name="work", bufs=1))
    acc = ctx.enter_context(tc.tile_pool(name="acc", bufs=1))
    ps = ctx.enter_context(tc.tile_pool(name="ps", bufs=2, space="PSUM"))
    ps1 = ctx.enter_context(tc.tile_pool(name="ps1", bufs=2, space="PSUM"))

    identf = const.tile([16, 16], F32)
    masks.make_identity(nc, identf[:])
    ident = const.tile([16, 16], F32R)
    nc.vector.tensor_copy(ident[:], identf[:, :])

    # ---- stage A: x in, means, xf0 ----
    xs = []
    sums = acc.tile([P, BPC * CT], F32R)
    xf0 = acc.tile([P, BPC * CT], BF16)             # mean-token cols (b, j)
    scratch = work.tile([P, S2], F32R, tag="scr")
    xpairs = []
    for pr in range(BPC // 2):
        xp2 = xres.tile([P, 2, CT, S2 + 2], BF16, tag=f"xp{pr}")
        nc.sync.dma_start(
            xp2[:, :, :, 0:S2],
            d["x"].ap()[2 * pr:2 * pr + 2].rearrange(
                "b (j p) s -> p (b j) s", p=P).rearrange(
                "p (b j) s -> p b j s", b=2))
        nc.vector.tensor_scalar_mul(xp2[:, :, :, S2 + 1:S2 + 2],
                                     xp2[:, :, :, 0:1], 0.0)
        xpairs.append(xp2)
    for b in range(BPC):
        xb = xpairs[b // 2][:, b % 2]
        xs.append(xb)

    # ---- weights needed early (after x DMAs in queue order) ----
    wqt = wbig.tile([P, CT, C], BF16, tag="wbig")   # W_q^T  (c-part, q)
    nc.sync.dma_start(wqt[:], d["wqt"].ap().rearrange("(j p) q -> p j q", p=P))
    wk = wbig.tile([P, CT, C], BF16, tag="wbig")    # W_k natural (krow-part, c)
    nc.sync.dma_start(wk[:], d["wk"].ap().rearrange("(t k) c -> k t c", k=P))
    posn = wsml.tile([P, CT, L], BF16)              # permuted pos, natural
    nc.sync.dma_start(posn[:], d["posn"].ap().rearrange("(j p) s -> p j s", p=P))
    post = wsml.tile([P, 2, C], BF16)               # spatial pos, transposed
    nc.sync.dma_start(post[:], d["post"].ap().rearrange("(t p) c -> p t c", p=P))
    posc = wsml.tile([1, C], BF16)                  # pos0 - mean_sp(pos)
    nc.sync.dma_start(posc[:], d["posc"].ap())
    bq = wsml.tile([P, CT], F32R)
    nc.sync.dma_start(bq[:], d["bq"].ap().rearrange("(j p) -> p j", p=P))
    bv = wsml.tile([P, CT], F32R)
    nc.sync.dma_start(bv[:], d["bv"].ap().rearrange("(j p) -> p j", p=P))
    bcn = wsml.tile([P, CT], F32R)
    nc.sync.dma_start(bcn[:], d["bc"].ap().rearrange("(j p) -> p j", p=P))
    wvt = wbf.tile([P, CT, C], BF16, tag="wv")      # W_v^T (c-part, vch)
    nc.sync.dma_start(wvt[:], d["wvt"].ap().rearrange("(j p) v -> p j v", p=P))
    wct = wbf.tile([P, CT, C], BF16, tag="wc")      # w_c^T (vch-part, o)
    nc.sync.dma_start(wct[:], d["wct"].ap().rearrange("(r p) o -> p r o", p=P))

    for b in range(BPC):
        xb = xs[b]
        for j in range(CT):
            if j % 2 == 0:
                nc.vector.reduce_sum(sums[:, b * CT + j:b * CT + j + 1],
                                     xb[:, j, 0:S2], axis=mybir.AxisListType.X)
            else:
                nc.scalar.activation(scratch[:], xb[:, j, 0:S2], AF.Copy,
                                     accum_out=sums[:, b * CT + j:b * CT + j + 1])
        for j in range(CT):
            nc.scalar.activation(xf0[:, b * CT + j:b * CT + j + 1],
                                 sums[:, b * CT + j:b * CT + j + 1], AF.Identity,
                                 bias=posn[:, j, S2:S2 + 1], scale=1.0 / S2)
            nc.scalar.activation(xb[:, j, S2:S2 + 1],
                                 sums[:, b * CT + j:b * CT + j + 1], AF.Identity,
                                 bias=posn[:, j, S2:S2 + 1], scale=1.0 / S2)

    # ---- stage B: q0 (batched over b) ----
    q0f = ps1.tile([P, P], F32, tag="seq")
    q0p = q0f[:, 0:CT * BPC]        # (q-part, (i, b))
    for i in range(CT):
        for j in range(CT):
            nc.tensor.matmul(q0p[:, i * BPC:(i + 1) * BPC],
                             wqt[:, j, i * P:(i + 1) * P],
                             xf0[:, b0j(j)],
                             start=(j == 0), stop=(j == CT - 1))
    # block-diagonal q0 (+bias) for the per-head W_k^T fold
    q0blk = acc.tile([P, CT * 16], BF16)
    nc.vector.memset(q0blk[:], 0.0)
    for i in range(CT):
        nc.scalar.activation(q0blk[0:64, i * 16:i * 16 + 8],
                             q0p[0:64, i * BPC:i * BPC + 8], AF.Identity,
                             bias=bq[0:64, i:i + 1])
        nc.scalar.activation(q0blk[64:P, i * 16 + 8:i * 16 + 16],
                             q0p[64:P, i * BPC:i * BPC + 8], AF.Identity,
                             bias=bq[64:P, i:i + 1])

    # ---- stage C: u = blockdiag(W_k)^T q0, scaled ----
    usb = acc.tile([P, CT * P], BF16)               # (c-part, (j, h, b))
    for j in range(CT):
        up = ps1.tile([P, P], F32, tag="seq")
        for t in range(CT):
            nc.tensor.matmul(up[:, t * 16:(t + 1) * 16],
                             wk[:, t, j * P:(j + 1) * P],
                             q0blk[:, t * 16:(t + 1) * 16])
        nc.vector.tensor_scalar_mul(usb[:, j * P:(j + 1) * P], up[:, :], SCALE2)

    # ---- per-batch: logits, softmax, w' transposes, y_x ----
    xtall = xtp.tile([P, 2 * BPC, C], BF16)
    nc.sync.dma_start(xtall[:], d["xt"].ap().rearrange(
        "b (t p) c -> p (b t) c", p=P))
    wta = acc.tile([P, 3 * P], BF16)                # w'^T batched (s-part,(t,h,b))
    yall = acc.tile([P, CT * P], BF16)              # y (c-part, (j, h, b))
    ypsb = acc.tile([P, CT * P], BF16)              # y_pos (c-part, (j, h, b))
    for b in range(BPC):
        lg = ps.tile([16, S2 + 2], F32, tag="lg")
        ub = [usb[:, j * P + b: (j + 1) * P: 8] for j in range(CT)]
        for j in range(CT):
            nc.tensor.matmul(lg[:, 0:S2 + 2], ub[j], xs[b][:, j, :],
                             start=(j == 0), stop=False)
        for j in range(CT):
            nc.tensor.matmul(lg[:, 0:S2], ub[j], posn[:, j, 0:S2],
                             start=False, stop=(j == CT - 1))
        # softmax over 257
        mx = work.tile([16, 4], F32, tag="mx")
        nc.vector.reduce_max(mx[:, 0:1], lg[:, 0:L], axis=mybir.AxisListType.X,
                             negate=True)
        ex = work.tile([16, L], F32R, tag="ex")
        nc.scalar.activation(ex[:, :], lg[:, 0:L], AF.Exp, bias=mx[:, 0:1],
                             accum_out=mx[:, 1:2])
        nc.vector.reciprocal(mx[:, 2:3], mx[:, 1:2])
        # w' = (e_sp + e_m/256) * r ; wm = e_m * r
        wp = work.tile([16, L], F32R, tag="wp")
        nc.vector.tensor_scalar_mul(mx[:, 3:4], ex[:, S2:S2 + 1], 1.0 / S2)
        nc.vector.tensor_scalar(wp[:, 0:S2], ex[:, 0:S2], mx[:, 3:4], mx[:, 2:3],
                                op0=mybir.AluOpType.add,
                                op1=mybir.AluOpType.mult)
        nc.vector.tensor_scalar(wp[:, S2:L], ex[:, S2:L], mx[:, 2:3], None,
                                op0=mybir.AluOpType.mult)
        # transpose w' -> (s-part, h) chunks; third chunk = wm row
        wtp = ps.tile([P, 48], F32R, tag="wt")
        nc.tensor.transpose(wtp[:, 0:16], wp[:, 0:P],
                            ident[:, :])
        nc.tensor.transpose(wtp[:, 16:32], wp[:, P:S2],
                            ident[:, :])
        nc.tensor.transpose(wtp[0:1, 32:48], wp[:, S2:L],
                            ident[:, :])
        for t in range(2):
            nc.vector.tensor_copy(wta[:, t * P + b:(t + 1) * P:8],
                                  wtp[:, t * 16:(t + 1) * 16])
        nc.vector.tensor_copy(wta[0:1, 2 * P + b:3 * P:8], wtp[0:1, 32:48])
        # y_x: stationary x^T tiles, moving w'^T
        yp = ps.tile([P, P], F32, tag="y")
        for j in range(CT):
            for t in range(2):
                nc.tensor.matmul(yp[:, j * 16:(j + 1) * 16],
                                 xtall[:, 2 * b + t, j * P:(j + 1) * P],
                                 wta[:, t * P + b:(t + 1) * P:8],
                                 start=(t == 0), stop=(t == 1))
        # scatter y_b into (j, h, b) layout: stride-8 columns for batch b
        nc.vector.tensor_copy(yall[:, b::8], yp[:, :])

    # ---- y_pos batched: pos^T against all-b w'^T ----
    for j in range(CT):
        ypp = ps1.tile([P, P], F32, tag="seq")
        for t in range(2):
            nc.tensor.matmul(ypp[:, :], post[:, t, j * P:(j + 1) * P],
                             wta[:, t * P:(t + 1) * P], start=(t == 0), stop=False)
        nc.tensor.matmul(ypp[:, :], posc[0:1, j * P:(j + 1) * P],
                         wta[0:1, 2 * P:3 * P], start=False, stop=True)
        nc.vector.tensor_copy(ypsb[:, j * P:(j + 1) * P], ypp[:, :])
    yfin = acc.tile([P, CT * P], BF16)
    nc.vector.tensor_add(yfin[:, :], yall[:, :], ypsb[:, :])

    # ---- a0 = blockdiag(W_v) y  (+ b_v) ----
    a0p = ps1.tile([P, P], F32, tag="seq")
    for r in range(CT):
        for j in range(CT):
            nc.tensor.matmul(a0p[:, r * 16:(r + 1) * 16],
                             wvt[:, j, r * P:(r + 1) * P],
                             yfin[:, j * P + 2 * r * 8: j * P + 2 * r * 8 + 16],
                             start=(j == 0), stop=(j == CT - 1))
    a0 = acc.tile([P, CT * BPC], BF16)              # (vch-part, (r, b))
    for r in range(CT):
        nc.scalar.activation(a0[0:64, r * 8:(r + 1) * 8],
                             a0p[0:64, r * 16:r * 16 + 8], AF.Identity,
                             bias=bv[0:64, r:r + 1])
        nc.scalar.activation(a0[64:P, r * 8:(r + 1) * 8],
                             a0p[64:P, r * 16 + 8:(r + 1) * 16], AF.Identity,
                             bias=bv[64:P, r:r + 1])

    # ---- out = w_c a0 + b_c ----
    opf = ps1.tile([P, P], F32, tag="seq")
    op = opf[:, 0:CT * BPC]
    for i in range(CT):
        for r in range(CT):
            nc.tensor.matmul(op[:, i * BPC:(i + 1) * BPC],
                             wct[:, r, i * P:(i + 1) * P],
                             a0[:, r * BPC:(r + 1) * BPC],
                             start=(r == 0), stop=(r == CT - 1))
    osb = acc.tile([P, CT * BPC], F32)
    for i in range(CT):
        nc.scalar.activation(osb[:, i * BPC:(i + 1) * BPC],
                             op[:, i * BPC:(i + 1) * BPC], AF.Identity,
                             bias=bcn[:, i:i + 1])
    nc.sync.dma_start(d["out"].ap(), osb[:])


def b0j(j):
    # xf0 columns for all b at fixed j: (b, j) layout -> stride CT
    return slice(j, BPC * CT, CT)


_CACHE = {}


def _get_nc():
    if "nc" in _CACHE:
        return _CACHE["nc"]
    nc = bacc.Bacc("TRN2", target_bir_lowering=False, debug=False,
                   num_devices=NCORE)
    d = {}
    d["x"] = nc.dram_tensor("x", [BPC, C, S2], BF16, kind="ExternalInput")
    d["xt"] = nc.dram_tensor("xt", [BPC, S2, C], BF16, kind="ExternalInput")
    d["posn"] = nc.dram_tensor("posn", [C, L], BF16, kind="ExternalInput")
    d["post"] = nc.dram_tensor("post", [S2, C], BF16, kind="ExternalInput")
    d["posc"] = nc.dram_tensor("posc", [1, C], BF16, kind="ExternalInput")
    d["wqt"] = nc.dram_tensor("wqt", [C, C], BF16, kind="ExternalInput")
    d["wk"] = nc.dram_tensor("wk", [C, C], BF16, kind="ExternalInput")
    d["wvt"] = nc.dram_tensor("wvt", [C, C], BF16, kind="ExternalInput")
    d["wct"] = nc.dram_tensor("wct", [C, C], BF16, kind="ExternalInput")
    d["bq"] = nc.dram_tensor("bq", [C], F32R, kind="ExternalInput")
    d["bv"] = nc.dram_tensor("bv", [C], F32R, kind="ExternalInput")
    d["bc"] = nc.dram_tensor("bc", [C], F32R, kind="ExternalInput")
    d["out"] = nc.dram_tensor("out", [P, CT * BPC], F32, kind="ExternalOutput")
    with tile.TileContext(nc) as tc, ExitStack() as ctx, \
            nc.allow_low_precision(reason="float32r tiles hold f32 bits"):
        _body(ctx, tc, d)
    nc.compile()
    _CACHE["nc"] = nc
    return nc


def _prep_maps(inputs):
    xf32 = inputs["x"].reshape(B, C, S2).astype(np.float32)
    x = np.ascontiguousarray(xf32).astype(ml_dtypes.bfloat16)
    xt = np.ascontiguousarray(xf32.transpose(0, 2, 1)).astype(ml_dtypes.bfloat16)
    pos = inputs["pos_emb"].astype(np.float32)
    posn = np.ascontiguousarray(np.concatenate([pos[:, 1:], pos[:, :1]], axis=1)).astype(ml_dtypes.bfloat16)
    post = np.ascontiguousarray(pos[:, 1:].T).astype(ml_dtypes.bfloat16)
    posc = np.ascontiguousarray((pos[:, 0] - pos[:, 1:].mean(axis=1))[None, :]
                                ).astype(ml_dtypes.bfloat16)
    wqkv = inputs["w_qkv"].astype(np.float32)
    wqt = np.ascontiguousarray(wqkv[0:C].T).astype(ml_dtypes.bfloat16)
    wk = np.ascontiguousarray(wqkv[C:2 * C]).astype(ml_dtypes.bfloat16)
    wvt = np.ascontiguousarray(wqkv[2 * C:3 * C].T).astype(ml_dtypes.bfloat16)
    wct = np.ascontiguousarray(inputs["w_c"].astype(np.float32).T).astype(ml_dtypes.bfloat16)
    bqkv = inputs["b_qkv"].astype(np.float32)
    shared = dict(posn=posn, post=post, posc=posc, wqt=wqt, wk=wk, wvt=wvt,
                  wct=wct, bq=np.ascontiguousarray(bqkv[0:C]),
                  bv=np.ascontiguousarray(bqkv[2 * C:3 * C]),
                  bc=inputs["b_c"].astype(np.float32))
    maps = []
    for c in range(NCORE):
        m = dict(shared)
        m["x"] = np.ascontiguousarray(x[c * BPC:(c + 1) * BPC])
        m["xt"] = np.ascontiguousarray(xt[c * BPC:(c + 1) * BPC])
        maps.append(m)
    return maps


def kernel(**inputs) -> np.ndarray:
    nc = _get_nc()
    maps = _prep_maps(inputs)
    res = run_bass_kernel_spmd(nc, maps, list(range(NCORE)))
    outs = []
    for c in range(NCORE):
        arr = res.results[c]["out"].reshape(P, CT, BPC)
        outs.append(arr.transpose(2, 1, 0).reshape(BPC, C))
    return np.concatenate(outs, axis=0).astype(np.float32)


if __name__ == "__main__":
    rng = np.random.default_rng(0)
    ins = {
        "x": rng.standard_normal((B, C, 16, 16), dtype=np.float32),
        "pos_emb": rng.standard_normal((C, L), dtype=np.float32) / 32,
        "w_qkv": rng.standard_normal((3 * C, C), dtype=np.float32) / 32,
        "b_qkv": rng.standard_normal((3 * C,), dtype=np.float32) * 0.1,
        "w_c": rng.standard_normal((C, C), dtype=np.float32) / 32,
        "b_c": rng.standard_normal((C,), dtype=np.float32) * 0.1,
    }
    o = kernel(**ins)
    print("out", o.shape, o.dtype, float(np.abs(o).mean()))



# revision 15
# speedup vs baseline: 1.6625x; 1.6625x over previous
"""AttentionPool2d Trainium2 kernel, 8-core batch-data-parallel, v2.

Math (reference returns only query position 0):
  xf_sp = x + pos_sp (fused on host, bf16)        [c, 256] per batch
  xf_m  = mean_s(xf_sp) + posc                    (posc = pos0 - mean(pos_sp))
  q0 = W_q xf_m + b_q   (only query needed; computed transposed: q0T[b, q])
  u  = blockdiag(W_k)^T q0, scaled by 1/8         (k never materialized)
  lg[b*16+h, s] = u_.^T xf ; batched softmax over all 128 (b,h) rows
  w' = w_sp + w_m/256 (mean token absorbed)
  yT[(b,h), c] = w_m*posc^T + sum_t w'^T xt       (computed transposed, then
                                                   PE-transposed back per j)
  a0 = blockdiag(W_v) y + b_v ; outT[b, o] = a0^T W_c^T + b_c
Host does all layout transforms; every DMA is partition-major contiguous.
"""
import sys
sys.path.insert(0, "/opt/trn_rl_repo")
import numpy as np
import ml_dtypes
from contextlib import ExitStack

from concourse import bacc, tile, mybir
import concourse.bass as bass
from concourse import masks
from concourse.bass_utils import run_bass_kernel_spmd

P = 128
B, C, S2, L = 64, 1024, 256, 257
XW = 258                           # xn row width (256 spatial + mean + pad)
NH = 16
NCORE, BPC, CT = 8, 8, 8           # cores, batches/core, c-tiles
F32R = mybir.dt.float32r
F32 = mybir.dt.float32
BF16 = mybir.dt.bfloat16
AF = mybir.ActivationFunctionType
AX = mybir.AxisListType
OP = mybir.AluOpType
SCALE2 = 1.0 / 8.0                 # (1/ch^0.25)^2 folded into u


def _body(ctx: ExitStack, tc, d):
    nc = tc.nc
    const = ctx.enter_context(tc.tile_pool(name="const", bufs=1))
    xbig = ctx.enter_context(tc.tile_pool(name="xbig", bufs=1))
    xtbig = ctx.enter_context(tc.tile_pool(name="xtbig", bufs=1))
    wpool = ctx.enter_context(tc.tile_pool(name="wpool", bufs=1))
    work = ctx.enter_context(tc.tile_pool(name="work", bufs=1))
    ps = ctx.enter_context(tc.tile_pool(name="ps", bufs=1, space="PSUM"))

    ident8 = const.tile([8, 8], BF16)
    masks.make_identity(nc, ident8[:])
    ident128 = const.tile([P, P], BF16)
    masks.make_identity(nc, ident128[:])
    ones8 = const.tile([1, 8], BF16)
    nc.gpsimd.memset(ones8[:], 1.0)

    # ---- small inputs ----
    posc_col = const.tile([P, 8], F32)
    nc.sync.dma_start(posc_col[:], d["posc_col"].ap())
    posc_row = const.tile([1, C], BF16)
    nc.sync.dma_start(posc_row[:], d["posc_row"].ap())
    bq_row = const.tile([1, C], BF16)
    nc.sync.dma_start(bq_row[:], d["bq_row"].ap())
    bc_row = const.tile([1, C], BF16)
    nc.sync.dma_start(bc_row[:], d["bc_row"].ap())
    bv_col = const.tile([P, 8], F32)
    nc.sync.dma_start(bv_col[:], d["bv_col"].ap())

    # ---- big DMAs, issue order = arrival order (HWDGE FIFO) ----
    xn = xbig.tile([P, BPC, CT, XW], BF16)
    for c in range(4):
        nc.sync.dma_start(xn[:, 2 * c:2 * c + 2], d["xn"].ap()[:, 2 * c:2 * c + 2])
    wqt = wpool.tile([P, CT, C], BF16, tag="wqt")
    for h in range(2):
        nc.sync.dma_start(wqt[:, 4 * h:4 * h + 4], d["wqt"].ap()[:, 4 * h:4 * h + 4])
    wk = wpool.tile([P, CT, C], BF16, tag="wk")
    for h in range(2):
        nc.sync.dma_start(wk[:, 4 * h:4 * h + 4], d["wk"].ap()[:, 4 * h:4 * h + 4])
    xt = xtbig.tile([P, BPC, 2, C], BF16)
    for c in range(4):
        nc.sync.dma_start(xt[:, 2 * c:2 * c + 2], d["xt"].ap()[:, 2 * c:2 * c + 2])
    wvt = wpool.tile([P, CT, C], BF16, tag="wvt")
    nc.sync.dma_start(wvt[:], d["wvt"].ap())
    wct = wpool.tile([P, CT, C], BF16, tag="wct")
    nc.sync.dma_start(wct[:], d["wct"].ap())

    # ---- means (chase xn chunks; split DVE/ACT/Pool) ----
    sums = work.tile([P, BPC * CT], F32)            # col = b*8 + j
    scr = work.tile([P, S2], F32R, tag="scr")
    for c in range(4):
        for b in (2 * c, 2 * c + 1):
            for j in range(5):
                nc.vector.reduce_sum(sums[:, b * CT + j:b * CT + j + 1],
                                     xn[:, b, j, 0:S2], axis=AX.X)
            for j in (5, 6, 7):
                nc.scalar.activation(scr[:], xn[:, b, j, 0:S2], AF.Copy,
                                     accum_out=sums[:, b * CT + j:b * CT + j + 1])
    # xf_m written into xn col 256:  mean/256 + posc
    for j in range(CT):
        nc.vector.tensor_scalar(xn[:, :, j, S2], sums[:, j:BPC * CT:CT],
                                1.0 / S2, posc_col[:, j:j + 1],
                                op0=OP.mult, op1=OP.add)

    # ---- q0T[b, q] = xf_m^T wqt + bq ----
    q0T = ps.tile([BPC, 2, 512], F32, tag="A")
    for h in range(2):
        nc.tensor.matmul(q0T[0:BPC, h, :], ones8[0:1, :],
                         bq_row[0:1, 512 * h:512 * (h + 1)],
                         start=True, stop=False)
    for j in range(CT):
        for h in range(2):
            nc.tensor.matmul(q0T[0:BPC, h, :], xn[:, :, j, S2],
                             wqt[:, j, 512 * h:512 * (h + 1)],
                             start=False, stop=(j == CT - 1))
    q0Tsb = work.tile([BPC, 2, 512], BF16)
    nc.scalar.activation(q0Tsb[:], q0T[0:BPC, :, :], AF.Copy)
    # transpose to (q-part, b), build block-diagonal per t
    q0p = ps.tile([P, 64], BF16, tag="B")
    for t in range(CT):
        nc.tensor.transpose(q0p[:, t * 8:(t + 1) * 8],
                            q0Tsb[0:BPC, t // 4, (t % 4) * P:(t % 4 + 1) * P],
                            ident8[:, :])
    q0blk = work.tile([P, CT, 16], BF16)
    nc.vector.memset(q0blk[:], 0.0)
    for t in range(CT):
        nc.vector.tensor_copy(q0blk[0:64, t, 0:8], q0p[0:64, t * 8:(t + 1) * 8])
        nc.vector.tensor_copy(q0blk[64:P, t, 8:16], q0p[64:P, t * 8:(t + 1) * 8])

    # ---- u[c, h*8+b] = blockdiag(W_k)^T q0, scaled ----
    ups = ps.tile([P, CT, P], F32, tag="A")
    for t in range(CT):
        for j in range(CT):
            nc.tensor.matmul(ups[:, j, t * 16:(t + 1) * 16],
                             wk[:, t, j * P:(j + 1) * P], q0blk[:, t, :],
                             start=True, stop=True)
    u_sb = work.tile([P, CT, P], BF16)
    for j in range(CT):
        if j % 2 == 0:
            nc.vector.tensor_scalar_mul(u_sb[:, j, :], ups[:, j, :], SCALE2)
        else:
            nc.scalar.activation(u_sb[:, j, :], ups[:, j, :], AF.Copy,
                                 scale=SCALE2)

    # ---- logits: row = (b%4)*32 + h, group g = b//4 (PE 32-align rule) ----
    lgs = [ps.tile([P, L], F32, tag=("C", "D")[g], name=f"lg{g}") for g in range(2)]
    for b in range(BPC):
        g, o = b // 4, (b % 4) * 32
        for j in range(CT):
            nc.tensor.matmul(lgs[g][o:o + 16, 0:L],
                             u_sb[:, j, b:P:8], xn[:, b, j, 0:L],
                             start=(j == 0), stop=(j == CT - 1),
                             tile_position=(0, o))

    # ---- batched softmax + w' (per group; pad rows carry garbage) ----
    mx = work.tile([P, 2, 4], F32, tag="mx")
    ex = work.tile([P, 2, L], F32R, tag="ex")
    wp = work.tile([P, 2, S2], BF16, tag="wp")
    wm = work.tile([P, 2, 1], BF16, tag="wm")
    for g in range(2):
        nc.vector.reduce_max(mx[:, g, 0:1], lgs[g][0:P, 0:L], axis=AX.X,
                             negate=True)
        nc.scalar.activation(ex[:, g, :], lgs[g][0:P, 0:L], AF.Exp,
                             bias=mx[:, g, 0:1], accum_out=mx[:, g, 1:2])
        nc.vector.reciprocal(mx[:, g, 2:3], mx[:, g, 1:2])
        nc.vector.tensor_scalar_mul(mx[:, g, 3:4], ex[:, g, S2:L], 1.0 / S2)
        nc.vector.tensor_scalar(wp[:, g, :], ex[:, g, 0:S2], mx[:, g, 3:4],
                                mx[:, g, 2:3], op0=OP.add, op1=OP.mult)
        nc.vector.tensor_scalar(wm[:, g, :], ex[:, g, S2:L], mx[:, g, 2:3],
                                None, op0=OP.mult)
    wtp = ps.tile([P, 2, 2, P], BF16, tag="E")
    wmp = ps.tile([1, 2, P], BF16, tag="F")
    for g in range(2):
        nc.tensor.transpose(wtp[:, g, 0, :], wp[:, g, 0:P], ident128[:, :])
        nc.tensor.transpose(wtp[:, g, 1, :], wp[:, g, P:S2], ident128[:, :])
        nc.tensor.transpose(wmp[0:1, g, :], wm[:, g, :], ident128[:, :])
    wta = work.tile([P, 2, 2, P], BF16)
    nc.vector.tensor_copy(wta[:], wtp[:, :, :, :])
    wmr = work.tile([1, 2, P], BF16)
    nc.vector.tensor_copy(wmr[:], wmp[0:1, :, :])

    # ---- yT[(b,h), c] = wm posc^T + sum_t w'^T xt (two groups) ----
    yTs = [ps.tile([P, 2, 512], F32, tag=("A", "B")[g], name=f"yT{g}") for g in range(2)]
    for g in range(2):
        for h in range(2):
            nc.tensor.matmul(yTs[g][:, h, :], wmr[0:1, g, :],
                             posc_row[0:1, 512 * h:512 * (h + 1)],
                             start=True, stop=False)
    for b in range(BPC):
        g, o = b // 4, (b % 4) * 32
        for t in range(2):
            for h in range(2):
                nc.tensor.matmul(yTs[g][o:o + 16, h, :],
                                 wta[:, g, t, o:o + 16],
                                 xt[:, b, t, 512 * h:512 * (h + 1)],
                                 start=False, stop=(t == 1),
                                 tile_position=(0, o))
    ysbT = work.tile([P, 2, 2, 512], BF16)
    for g in range(2):
        nc.scalar.activation(ysbT[:, g, 0, :], yTs[g][:, 0, :], AF.Copy)
        nc.scalar.activation(ysbT[:, g, 1, :], yTs[g][:, 1, :], AF.Copy)

    # ---- per j: transpose yT -> y_sb[c, h*8+b], then a0 ----
    y_sb = work.tile([P, CT, 16, BPC], BF16)        # col = h*8 + (g*4+boff)
    a0ps = ps.tile([P, CT, 16], F32, tag="F")
    for j in range(CT):
        yps = ps.tile([P, 2, P], BF16, tag=("C", "D")[j % 2], name=f"yps{j}")
        for g in range(2):
            nc.tensor.transpose(yps[:, g, :],
                                ysbT[:, g, j // 4, (j % 4) * P:(j % 4 + 1) * P],
                                ident128[:, :])
        for g in range(2):
            # in cols (boff, h) of yps; out col h*8 + g*4 + boff
            src = yps[:, g, 0:P].rearrange("p (b h) -> p b h", b=4)[:, :, 0:16]
            if j % 2 == 0:
                nc.vector.tensor_copy(
                    y_sb[:, j, :, g * 4:g * 4 + 4].rearrange(
                        "p h b -> p b h"), src)
            else:
                nc.scalar.activation(
                    y_sb[:, j, :, g * 4:g * 4 + 4].rearrange(
                        "p h b -> p b h"), src, AF.Copy)
    for r in range(CT):
        for j in range(CT):
            nc.tensor.matmul(a0ps[:, r, :], wvt[:, j, r * P:(r + 1) * P],
                             y_sb[:, j, 2 * r:2 * r + 2, :],
                             start=(j == 0), stop=(j == CT - 1))
    a0sb = work.tile([P, CT, BPC], BF16)
    for r in range(CT):
        nc.vector.tensor_scalar(a0sb[0:64, r, :], a0ps[0:64, r, 0:8],
                                bv_col[0:64, r:r + 1], None, op0=OP.add)
        nc.vector.tensor_scalar(a0sb[64:P, r, :], a0ps[64:P, r, 8:16],
                                bv_col[64:P, r:r + 1], None, op0=OP.add)

    # ---- outT[b, o] = a0^T wct + bc ----
    outT = ps.tile([BPC, 2, 512], F32, tag="A")
    for h in range(2):
        nc.tensor.matmul(outT[0:BPC, h, :], ones8[0:1, :],
                         bc_row[0:1, 512 * h:512 * (h + 1)],
                         start=True, stop=False)
    for r in range(CT):
        for h in range(2):
            nc.tensor.matmul(outT[0:BPC, h, :], a0sb[:, r, :],
                             wct[:, r, 512 * h:512 * (h + 1)],
                             start=False, stop=(r == CT - 1))
    osb = work.tile([BPC, 2, 512], F32)
    nc.scalar.activation(osb[:], outT[0:BPC, :, :], AF.Copy)
    nc.sync.dma_start(d["out"].ap(), osb[0:BPC, :, :])
    if "dsums" in d:
        nc.sync.dma_start(d["dsums"].ap(), sums[:])
        nc.sync.dma_start(d["dq0T"].ap(), q0Tsb[:])
        nc.sync.dma_start(d["dusb"].ap(), u_sb[:])
        nc.sync.dma_start(d["dwp"].ap(), wp[:])
        nc.sync.dma_start(d["dwm"].ap(), wm[:])
        nc.sync.dma_start(d["dysb"].ap(), y_sb[:])
        nc.sync.dma_start(d["da0"].ap(), a0sb[:])
        nc.sync.dma_start(d["dex"].ap(), ex[:])


_CACHE = {}


def _get_nc():
    if "nc" in _CACHE:
        return _CACHE["nc"]
    nc = bacc.Bacc("TRN2", target_bir_lowering=False, debug=False,
                   num_devices=NCORE)
    d = {}
    d["xn"] = nc.dram_tensor("xn", [P, BPC, CT, XW], BF16, kind="ExternalInput")
    d["xt"] = nc.dram_tensor("xt", [P, BPC, 2, C], BF16, kind="ExternalInput")
    d["wqt"] = nc.dram_tensor("wqt", [P, CT, C], BF16, kind="ExternalInput")
    d["wk"] = nc.dram_tensor("wk", [P, CT, C], BF16, kind="ExternalInput")
    d["wvt"] = nc.dram_tensor("wvt", [P, CT, C], BF16, kind="ExternalInput")
    d["wct"] = nc.dram_tensor("wct", [P, CT, C], BF16, kind="ExternalInput")
    d["posc_col"] = nc.dram_tensor("posc_col", [P, 8], F32, kind="ExternalInput")
    d["posc_row"] = nc.dram_tensor("posc_row", [1, C], BF16, kind="ExternalInput")
    d["bq_row"] = nc.dram_tensor("bq_row", [1, C], BF16, kind="ExternalInput")
    d["bc_row"] = nc.dram_tensor("bc_row", [1, C], BF16, kind="ExternalInput")
    d["bv_col"] = nc.dram_tensor("bv_col", [P, 8], F32, kind="ExternalInput")
    d["out"] = nc.dram_tensor("out", [BPC, C], F32, kind="ExternalOutput")
    import os
    if os.environ.get("KDBG"):
        d["dsums"] = nc.dram_tensor("dsums", [P, BPC * CT], F32, kind="ExternalOutput")
        d["dq0T"] = nc.dram_tensor("dq0T", [BPC, 2, 512], BF16, kind="ExternalOutput")
        d["dusb"] = nc.dram_tensor("dusb", [P, CT, P], BF16, kind="ExternalOutput")
        d["dwp"] = nc.dram_tensor("dwp", [P, 2, S2], BF16, kind="ExternalOutput")
        d["dwm"] = nc.dram_tensor("dwm", [P, 2, 1], BF16, kind="ExternalOutput")
        d["dysb"] = nc.dram_tensor("dysb", [P, CT, 16, BPC], BF16, kind="ExternalOutput")
        d["da0"] = nc.dram_tensor("da0", [P, CT, BPC], BF16, kind="ExternalOutput")
        d["dex"] = nc.dram_tensor("dex", [P, 2, L], F32R, kind="ExternalOutput")
    with tile.TileContext(nc) as tc, ExitStack() as ctx, \
            nc.allow_low_precision(reason="float32r tiles hold f32 bits"):
        _body(ctx, tc, d)
    nc.compile()
    _CACHE["nc"] = nc
    return nc


def _prep_maps(inputs):
    bf = ml_dtypes.bfloat16
    x = inputs["x"].reshape(B, C, S2).astype(np.float32)
    pos = inputs["pos_emb"].astype(np.float32)            # [C, 257]
    xf = x + pos[None, :, 1:]                             # [B, C, S2]
    posc = pos[:, 0] - pos[:, 1:].mean(axis=1)            # [C]
    wqkv = inputs["w_qkv"].astype(np.float32)
    wq, wkm, wv = wqkv[0:C], wqkv[C:2 * C], wqkv[2 * C:3 * C]
    wc = inputs["w_c"].astype(np.float32)
    bqkv = inputs["b_qkv"].astype(np.float32)

    def pmaj(m):  # [C, N] -> [128, 8, N] partition-major
        return np.ascontiguousarray(
            m.reshape(CT, P, -1).transpose(1, 0, 2)).astype(bf)

    shared = dict(
        wqt=pmaj(wq.T), wk=pmaj(wkm), wvt=pmaj(wv.T), wct=pmaj(wc.T),
        posc_col=np.ascontiguousarray(posc.reshape(CT, P).T).astype(np.float32),
        posc_row=np.ascontiguousarray(posc[None, :]).astype(bf),
        bq_row=np.ascontiguousarray(bqkv[0:C][None, :]).astype(bf),
        bc_row=np.ascontiguousarray(inputs["b_c"].astype(np.float32)[None, :]
                                    ).astype(bf),
        bv_col=np.ascontiguousarray(
            bqkv[2 * C:3 * C].reshape(CT, P).T).astype(np.float32),
    )
    maps = []
    for cb in range(NCORE):
        xc = xf[cb * BPC:(cb + 1) * BPC]                  # [8, C, S2]
        xnc = np.zeros((P, BPC, CT, XW), dtype=bf)
        xnc[:, :, :, 0:S2] = xc.reshape(BPC, CT, P, S2).transpose(2, 0, 1, 3
                                                                  ).astype(bf)
        xtc = np.ascontiguousarray(
            xc.reshape(BPC, C, 2, P).transpose(3, 0, 2, 1)).astype(bf)
        m = dict(shared)
        m["xn"] = np.ascontiguousarray(xnc)
        m["xt"] = xtc
        maps.append(m)
    return maps


def kernel(**inputs) -> np.ndarray:
    nc = _get_nc()
    maps = _prep_maps(inputs)
    res = run_bass_kernel_spmd(nc, maps, list(range(NCORE)))
    outs = [res.results[c]["out"].reshape(BPC, C) for c in range(NCORE)]
    return np.concatenate(outs, axis=0).astype(np.float32)


if __name__ == "__main__":
    rng = np.random.default_rng(0)
    ins = {
        "x": rng.standard_normal((B, C, 16, 16), dtype=np.float32),
        "pos_emb": rng.standard_normal((C, L), dtype=np.float32) / 32,
        "w_qkv": rng.standard_normal((3 * C, C), dtype=np.float32) / 32,
        "b_qkv": rng.standard_normal((3 * C,), dtype=np.float32) * 0.1,
        "w_c": rng.standard_normal((C, C), dtype=np.float32) / 32,
        "b_c": rng.standard_normal((C,), dtype=np.float32) * 0.1,
    }
    o = kernel(**ins)
    print("out", o.shape, o.dtype, float(np.abs(o).mean()))
